# revision 4
# baseline (speedup 1.0000x reference)
"""BitwiseMLP Trainium2 kernel: 8-way data-parallel over the batch dim.

Math (per reference):
  h0 = x @ W0.T + b0; h0 = BN0(h0); s0 = sign(h0)
  h1 = s0 @ sign(W1).T + b1; h1 = BN1(h1); s1 = sign(h1)
  out = (s1 @ sign(W2).T + b2) * out_scale

Device strategy (per core, batch shard of 8192 rows; activations stay
transposed [channel, batch] end-to-end so the device does zero transposes):
  - L0 main: x,W0 rounded to 12-bit significands (11 explicit bits -- the
    exact width the f32r operand path preserves, probed on hw) and
    multiplied at full PE rate via float32r.
  - L0 correction: the residual terms xh@Wl + xl@Wh are computed in
    fp8e4m3 with DoubleRow (2 contraction rows/pass), K=2048 packed into
    8 DR matmuls accumulating in a SEPARATE psum at net scale 2^17
    (operand pre-scales: xh*1 @ Wl*2^17, xl*2^13 @ Wh*2^4; all exact
    powers of two). This halves the correction cost vs fp16.
  - Merge: one DVE scalar_tensor_tensor per tile:
      h0 = corr_psum * 2^-17 + main_psum  -> SBUF
    then ScalarE Sign(h0*A + B) with the BN fold, output fp8e4 (+-1).
  - L1/L2 are exact +-1 fp8e4 matmuls with DoubleRow; results are small
    even integers accumulated exactly in fp32 PSUM.
  - Final eviction: Identity activation out = psum*out_scale + b2*out_scale.
  - DMAs are deadline-ordered: bt0's x tiles and per-m weight chunks
    arrive in consumption order so the PE starts ~8us in.
Host does the batch shard, the transposes and the hi/lo splits; the output
comes back transposed per core and is re-assembled in numpy.
"""
import os
import sys
import types

import numpy as np

import concourse.bass as bass
import concourse.mybir as mybir
import concourse.tile as tile
from concourse import bacc
from concourse.bass_utils import run_bass_kernel_spmd


def _ensure_axon_hooks():
    """concourse.bass_utils imports antenv.axon_hooks when tracing is
    requested (BASS_TRACE=1). The trimmed image lacks that module, which
    would turn an optional profile into a crash — synthesize it, wiring the
    real NTFF hook when libaxon_pjrt.so is present."""
    try:
        import antenv.axon_hooks  # noqa: F401
        return
    except ImportError:
        pass
    try:
        import antenv
    except ImportError:
        return
    mod = types.ModuleType("antenv.axon_hooks")
    state = {"hook": None}
    mod.set_axon_ntff_profile_hook = lambda h: state.update(hook=h)
    mod.get_axon_ntff_profile_hook = lambda: state["hook"]
    sys.modules["antenv.axon_hooks"] = mod
    antenv.axon_hooks = mod
    so = "/opt/axon/libaxon_pjrt.so"
    if os.path.exists(so):
        try:
            from trn_agent_boot.trn_boot import _ntff_profile_via_ctypes
            mod.set_axon_ntff_profile_hook(_ntff_profile_via_ctypes(so))
            import concourse.bass_utils as _bu
            _real_upload = _bu.upload_artifacts

            def _safe_upload(tmpdir):
                try:
                    return _real_upload(tmpdir)
                except Exception:
                    return f"local:{tmpdir}"

            _bu.upload_artifacts = _safe_upload
        except Exception:
            pass


_ensure_axon_hooks()

dt = mybir.dt
P = 128
D = 1024
B = 65536
NCORES = 8
BS = B // NCORES          # 8192 batch rows per core
BT = 512                  # batch-tile width (columns of transposed activations)
NBT = BS // BT            # 16 batch tiles per core
KO = D // P               # 8 k-subtiles of 128 channels
K2 = 2 * KO               # 16 fp8 correction blocks (8 xh + 8 xl)
EPS = 1e-5
S_WL = 17                 # Wl pre-scale exponent (net corr psum scale 2^17)
S_XL = 13                 # xl pre-scale exponent
S_WH = S_WL - S_XL        # Wh pre-scale exponent

LAST_RESULTS = None       # BassKernelResults of the most recent run (for profiling)
_NC = None                # cached compiled Bass module (build once per process)


def _round_sig12(a: np.ndarray) -> np.ndarray:
    """Round fp32 magnitudes to 12-bit significands (11 explicit mantissa
    bits), round-half-to-even. Values of this form pass through the PE's
    float32r operand read exactly (hw-probed: both operands keep exactly
    11 explicit bits)."""
    u = a.view(np.uint32).astype(np.uint64)
    half = np.uint64(1 << 11)
    one = np.uint64(1)
    r = (u + half - one + ((u >> np.uint64(12)) & one)) & ~np.uint64((1 << 12) - 1)
    return r.astype(np.uint32).view(np.float32)


def _build():
    nc = bacc.Bacc(num_devices=NCORES)
    # x tiles chunked per batch-tile for contiguous DMA lines
    xh = nc.dram_tensor("xh", [P, NBT, KO, BT], dt.float32r, kind="ExternalInput")
    x8 = nc.dram_tensor("x8", [P, NBT, K2, BT], dt.float8e4, kind="ExternalInput")
    # weights chunked per output block m
    w0h = nc.dram_tensor("w0h", [P, KO, KO, P], dt.float32r, kind="ExternalInput")
    w8 = nc.dram_tensor("w8", [P, KO, K2, P], dt.float8e4, kind="ExternalInput")
    w1 = nc.dram_tensor("w1", [P, KO, D], dt.float8e4, kind="ExternalInput")
    w2 = nc.dram_tensor("w2", [P, KO, D], dt.float8e4, kind="ExternalInput")
    vec = nc.dram_tensor("vec", [P, 6, KO], dt.float32, kind="ExternalInput")
    out = nc.dram_tensor("out", [P, KO, BS], dt.float32, kind="ExternalOutput")

    Sign = mybir.ActivationFunctionType.Sign
    Ident = mybir.ActivationFunctionType.Identity
    DR = mybir.MatmulPerfMode.DoubleRow
    Alu = mybir.AluOpType
    ts = bass.ts
    CDESC = float(2.0 ** -S_WL)

    with tile.TileContext(nc) as tc:
        with (
            tc.tile_pool(name="wpool", bufs=1) as wpool,
            tc.tile_pool(name="xpool", bufs=2) as xpool,
            tc.tile_pool(name="spool", bufs=2) as spool,
            tc.tile_pool(name="hpool", bufs=3) as hpool,
            tc.tile_pool(name="opool", bufs=3) as opool,
            tc.tile_pool(name="pspool", bufs=2, space="PSUM") as pspool,
        ):
            w0h_sb = wpool.tile([P, KO, KO, P], dt.float32r)
            w8_sb = wpool.tile([P, KO, K2, P], dt.float8e4)
            w1_sb = wpool.tile([P, KO, D], dt.float8e4)
            w2_sb = wpool.tile([P, KO, D], dt.float8e4)
            vec_sb = wpool.tile([P, 6, KO], dt.float32)

            # ---- deadline-ordered prologue DMAs ----
            # sync queue: w0h-m0 then bt0's xh in 2-k chunks (first mains
            # start after ~0.75 MiB); gpsimd queue in parallel: bt0's fp8
            # operands + vec + per-m w8.
            xh_sb0 = xpool.tile([P, KO, BT], dt.float32r, tag="xh")
            x8_sb0 = xpool.tile([P, K2, BT], dt.float8e4, tag="x8")
            nc.sync.dma_start(w0h_sb[:, 0], w0h[:, 0])
            for kk in range(4):
                nc.sync.dma_start(xh_sb0[:, 2 * kk:2 * kk + 2, :],
                                  xh[:, 0, 2 * kk:2 * kk + 2, :])
            nc.gpsimd.dma_start(x8_sb0, x8[:, 0])
            nc.gpsimd.dma_start(w8_sb[:, 0], w8[:, 0])
            nc.gpsimd.dma_start(vec_sb, vec[:])
            for m in range(1, KO):
                nc.sync.dma_start(w0h_sb[:, m], w0h[:, m])
                nc.gpsimd.dma_start(w8_sb[:, m], w8[:, m])
            nc.sync.dma_start(w1_sb, w1[:])
            nc.sync.dma_start(w2_sb, w2[:])

            def emit_L0(bt, xh_sb, x8_sb, s0_sb):
                for m in range(KO):
                    mps = pspool.tile([P, BT], dt.float32, tag="mps", bufs=2,
                                      name=f"mps_{bt}_{m}")
                    for k in range(KO):
                        nc.tensor.matmul(mps, w0h_sb[:, m, k, :],
                                         xh_sb[:, k, :],
                                         start=k == 0, stop=k == KO - 1)
                    cps = pspool.tile([P, BT], dt.float32, tag="cps", bufs=2,
                                      name=f"cps_{bt}_{m}")
                    for kp in range(KO):
                        nc.tensor.matmul(cps, w8_sb[:, m, 2 * kp:2 * kp + 2, :],
                                         x8_sb[:, 2 * kp:2 * kp + 2, :],
                                         start=kp == 0, stop=kp == KO - 1,
                                         perf_mode=DR)
                    c_sb = hpool.tile([P, BT], dt.float32, tag="c0",
                                      name=f"c0_{bt}_{m}")
                    nc.scalar.mul(c_sb, cps, CDESC)
                    h0_sb = hpool.tile([P, BT], dt.float32, tag="h0",
                                       name=f"h0_{bt}_{m}")
                    nc.vector.scalar_tensor_tensor(h0_sb, mps, 1.0, c_sb,
                                                   Alu.mult, Alu.add)
                    nc.scalar.activation(s0_sb[:, m, :], h0_sb, Sign,
                                         bias=vec_sb[:, 1, m:m + 1],
                                         scale=vec_sb[:, 0, m:m + 1])

            def emit_L1(s0_sb, s1_sb, bt):
                for m in range(KO):
                    ps = pspool.tile([P, BT], dt.float32, tag="ps1", bufs=2,
                                     name=f"ps1_{bt}_{m}")
                    for kp in range(KO // 2):
                        nc.tensor.matmul(ps, w1_sb[:, 2 * kp:2 * kp + 2, ts(m, P)],
                                         s0_sb[:, 2 * kp:2 * kp + 2, :],
                                         start=kp == 0, stop=kp == KO // 2 - 1,
                                         perf_mode=DR)
                    nc.scalar.activation(s1_sb[:, m, :], ps, Sign,
                                         bias=vec_sb[:, 3, m:m + 1],
                                         scale=vec_sb[:, 2, m:m + 1])

            def emit_L2(s1_sb, bt):
                sl = bass.ds(bt * BT, BT)
                for m in range(KO):
                    ps = pspool.tile([P, BT], dt.float32, tag="ps2", bufs=2,
                                     name=f"ps2_{bt}_{m}")
                    for kp in range(KO // 2):
                        nc.tensor.matmul(ps, w2_sb[:, 2 * kp:2 * kp + 2, ts(m, P)],
                                         s1_sb[:, 2 * kp:2 * kp + 2, :],
                                         start=kp == 0, stop=kp == KO // 2 - 1,
                                         perf_mode=DR)
                    o_sb = opool.tile([P, BT], dt.float32, tag="om",
                                      name=f"om_{bt}_{m}")
                    nc.scalar.activation(o_sb, ps, Ident,
                                         bias=vec_sb[:, 5, m:m + 1],
                                         scale=vec_sb[:, 4, m:m + 1])
                    nc.sync.dma_start(out[:, m, sl], o_sb)

            # software pipeline: iteration `it` runs L0(it), L1(it-1),
            # L2(it-2) so the sign-activation latency of each stage hides
            # behind a full tile of PE work from the next stage.
            s0_ring, s1_ring = {}, {}
            for it in range(NBT + 2):
                if it < NBT:
                    if it == 0:
                        xh_sb, x8_sb = xh_sb0, x8_sb0
                    else:
                        xh_sb = xpool.tile([P, KO, BT], dt.float32r, tag="xh",
                                           name=f"xh_{it}")
                        x8_sb = xpool.tile([P, K2, BT], dt.float8e4, tag="x8",
                                           name=f"x8_{it}")
                        nc.sync.dma_start(xh_sb, xh[:, it])
                        nc.sync.dma_start(x8_sb, x8[:, it])
                    s0_sb = spool.tile([P, KO, BT], dt.float8e4, tag="s0",
                                       name=f"s0_{it}")
                    emit_L0(it, xh_sb, x8_sb, s0_sb)
                    s0_ring[it] = s0_sb
                if 1 <= it <= NBT:
                    s1_sb = spool.tile([P, KO, BT], dt.float8e4, tag="s1",
                                       name=f"s1_{it - 1}")
                    emit_L1(s0_ring.pop(it - 1), s1_sb, it - 1)
                    s1_ring[it - 1] = s1_sb
                if it >= 2:
                    emit_L2(s1_ring.pop(it - 2), it - 2)

    nc.compile()
    return nc


def kernel(**inputs) -> np.ndarray:
    global LAST_RESULTS
    f32 = np.float32
    x = np.asarray(inputs["x"], f32)
    W0 = np.asarray(inputs["W0"], f32)
    b0 = np.asarray(inputs["b0"], f32)
    W1 = np.asarray(inputs["W1"], f32)
    b1 = np.asarray(inputs["b1"], f32)
    W2 = np.asarray(inputs["W2"], f32)
    b2 = np.asarray(inputs["b2"], f32)
    bn0_g = np.asarray(inputs["bn0_g"], f32)
    bn0_b = np.asarray(inputs["bn0_b"], f32)
    bn0_rm = np.asarray(inputs["bn0_rm"], f32)
    bn0_rv = np.asarray(inputs["bn0_rv"], f32)
    bn1_g = np.asarray(inputs["bn1_g"], f32)
    bn1_b = np.asarray(inputs["bn1_b"], f32)
    bn1_rm = np.asarray(inputs["bn1_rm"], f32)
    bn1_rv = np.asarray(inputs["bn1_rv"], f32)
    osc = np.asarray(inputs["out_scale"], f32)

    # per-channel affine folds (BN in eval mode):
    #   bn0(h+b0) = h*A0 + B0 ; bn1(h+b1) = h*A1 + B1 ; out = h*CS + CB
    inv0 = (bn0_g / np.sqrt(bn0_rv + EPS)).astype(f32)
    inv1 = (bn1_g / np.sqrt(bn1_rv + EPS)).astype(f32)
    A0, B0 = inv0, ((b0 - bn0_rm) * inv0 + bn0_b).astype(f32)
    A1, B1 = inv1, ((b1 - bn1_rm) * inv1 + bn1_b).astype(f32)
    CS, CB = osc, (b2 * osc).astype(f32)
    vec = np.stack([A0, B0, A1, B1, CS, CB])           # [6, D]
    vec_host = np.ascontiguousarray(
        vec.reshape(6, KO, P).transpose(2, 0, 1))      # [P, 6, KO]

    e4m3 = mybir.dt.np(dt.float8e4)

    def pm(a):
        # [cols, D] -> partition-major [P, KO, cols]
        return np.ascontiguousarray(a.T.reshape(KO, P, -1).transpose(1, 0, 2))

    # weights: 12-bit-significand main (f32r-exact) + e4m3 residuals
    W0h = _round_sig12(W0)
    W0l = W0 - W0h
    w0h_pm = pm(W0h)                                   # [P, KO(k), D(cols)]
    # chunk per output block m: [P, m, k, 128]
    w0h_host = np.ascontiguousarray(
        w0h_pm.reshape(P, KO, KO, P).transpose(0, 2, 1, 3))
    wl8_pm = pm((W0l * f32(2.0 ** S_WL)).astype(e4m3))
    wh8_pm = pm((W0h * f32(2.0 ** S_WH)).astype(e4m3))
    w8_full = np.concatenate([wl8_pm, wh8_pm], axis=1)  # [P, 16, D]
    w8_host = np.ascontiguousarray(
        w8_full.reshape(P, K2, KO, P).transpose(0, 2, 1, 3))
    w1_host = pm(np.sign(W1).astype(e4m3))
    w2_host = pm(np.sign(W2).astype(e4m3))

    # activations: f32r main + e4m3 residual pair, batch-tile chunked
    xh_full = _round_sig12(x)
    xl_full = x - xh_full
    xhT = pm(xh_full)                                  # [P, KO, B]
    xh8T = pm(xh_full.astype(e4m3))
    xl8T = pm((xl_full * f32(2.0 ** S_XL)).astype(e4m3))
    x8T = np.concatenate([xh8T, xl8T], axis=1)         # [P, 16, B]

    shared = {
        "w0h": w0h_host, "w8": w8_host,
        "w1": w1_host, "w2": w2_host, "vec": vec_host,
    }
    in_maps = []
    for c in range(NCORES):
        bs = slice(c * BS, (c + 1) * BS)
        xh_c = xhT[:, :, bs].reshape(P, KO, NBT, BT).transpose(0, 2, 1, 3)
        x8_c = x8T[:, :, bs].reshape(P, K2, NBT, BT).transpose(0, 2, 1, 3)
        in_maps.append({
            **shared,
            "xh": np.ascontiguousarray(xh_c),
            "x8": np.ascontiguousarray(x8_c),
        })

    global _NC
    if _NC is None:
        _NC = _build()
    res = run_bass_kernel_spmd(_NC, in_maps, core_ids=list(range(NCORES)))
    LAST_RESULTS = res

    out = np.empty((B, D), f32)
    for c in range(NCORES):
        # [P, KO, BS] -> [BS, KO*P] with channel = ko*P + p
        o = res.results[c]["out"].transpose(2, 1, 0).reshape(BS, D)
        out[c * BS:(c + 1) * BS] = o
    return out
